# revision 108
# baseline (speedup 1.0000x reference)
"""Sliding-window (banded causal) MHA on 8 TRN2 NeuronCores — v4.

Sharding: 8 cores = 2 batches x 4 head-groups (4 heads x 64 dims).
Baseline-proven interleaved attention loop, plus:
  - PE warmup: dummy matmuls from ~0.8us absorb the p-state ramp and
    the initial DMA latency (the ramp model halves throughput for the
    first ~3us of PE-busy otherwise).
  - fp8 DoubleRow projections: Q/K/V projections run as 3-term
    hi/lo-residual fp8 matmuls (Wh@xh + Wh@xl + Wl@xh accumulated in
    PSUM). K=256 per instruction at 0.5 cycles/row -> 0.75x the bf16
    row count for the same math, ~bf16 accuracy (hi=e4m3, lo=e5m2
    residuals; dropped Wl@xl term is ~0.13%).

Engine assignment (cost-model driven):
  PE    : projections, scores, PV, output projection
  Act   : exp ONLY (plus Q/K PSUM drains during the projection phase)
  DVE   : rope multiplies, probs edge masks, reciprocal, ctx drains,
          half the outproj drains
  Pool  : V drains, denominator partition-broadcast, other half of the
          outproj drains
  DMA   : loads, rope rotate-half shifts, stores (bf16)

Normalization: V carries a ones column, so PV emits raw ctx plus the
softmax sums (psum row 64). reciprocal (DVE, psum-in) -> partition_
broadcast (Pool) -> tensor_mul drains (DVE) fold the divide into the
PSUM drain. Biases: bq/bk are zero per the problem spec (asserted on
host); bv/bo are folded exactly into a host-side constant.
"""

from contextlib import ExitStack

import numpy as np
import ml_dtypes

import concourse.bass as bass
import concourse.tile as tile
from concourse import bacc, mybir
from concourse.bass_utils import run_bass_kernel_spmd

BF16 = mybir.dt.bfloat16
F32 = mybir.dt.float32
FP8H = mybir.dt.float8e4
FP8L = mybir.dt.float8e5

B, S, H = 2, 2048, 1024
NH, HD = 16, 64
WINDOW = 1024
ROPE_THETA = 10000.0
MAX_POS = 2048
N_CORES = 8
HG = 4                      # heads per core
GD = HG * HD                # 256
P = 128
NQT = S // P                # 16
WT = WINDOW // P            # 8
CH = H // P                 # 8
CP2 = CH // 2               # 4 chunk-pairs (K=256 per DoubleRow matmul)
VW = HD + 1                 # 65
WSCALE = 32.0               # host-side weight scale (fp8 subnormal guard)

_cache = {}


def _build():
    nc = bacc.Bacc("TRN2", target_bir_lowering=False, debug=False,
                   enable_asserts=False, num_devices=N_CORES)

    xh_d = nc.dram_tensor("xh", [H, S], FP8H, kind="ExternalInput")
    xl_d = nc.dram_tensor("xl", [H, S], FP8L, kind="ExternalInput")
    wqkh_d = nc.dram_tensor("wqkh", [H, 2 * GD], FP8H, kind="ExternalInput")
    wqkl_d = nc.dram_tensor("wqkl", [H, 2 * GD], FP8L, kind="ExternalInput")
    wvh_d = nc.dram_tensor("wvh", [H, GD], FP8H, kind="ExternalInput")
    wvl_d = nc.dram_tensor("wvl", [H, GD], FP8L, kind="ExternalInput")
    woT_d = nc.dram_tensor("woT", [GD, H], BF16, kind="ExternalInput")
    perm_d = nc.dram_tensor("perm", [P, P], BF16, kind="ExternalInput")
    cosT_d = nc.dram_tensor("cosT", [P, S], BF16, kind="ExternalInput")
    sinTs_d = nc.dram_tensor("sinTs", [P, S], BF16, kind="ExternalInput")
    masks_d = nc.dram_tensor("masks", [P, 3 * P], BF16, kind="ExternalInput")
    out_d = nc.dram_tensor("out", [S, H], BF16, kind="ExternalOutput")

    with tile.TileContext(nc) as tc, ExitStack() as ctx:
        const = ctx.enter_context(tc.tile_pool(name="const", bufs=1))
        qk = ctx.enter_context(tc.tile_pool(name="qk", bufs=1))
        vp = ctx.enter_context(tc.tile_pool(name="vp", bufs=1))
        pp = ctx.enter_context(tc.tile_pool(name="pp", bufs=20))
        cxp = ctx.enter_context(tc.tile_pool(name="cxp", bufs=1))
        sm = ctx.enter_context(tc.tile_pool(name="sm", bufs=10))

        warm = const.tile([1, 640], BF16, name="warm")
        wqkh_sb = const.tile([P, CH * 2 * GD], FP8H, name="wqkh_sb")
        wqkl_sb = const.tile([P, CH * 2 * GD], FP8L, name="wqkl_sb")
        wvh_sb = const.tile([P, CH * GD], FP8H, name="wvh_sb")
        wvl_sb = const.tile([P, CH * GD], FP8L, name="wvl_sb")
        wo_sb = const.tile([P, 2 * H], BF16, name="wo_sb")
        xh_sb = const.tile([P, CH * S], FP8H, name="xh_sb")
        xl_sb = const.tile([P, CH * S], FP8L, name="xl_sb")
        cosT = const.tile([P, S], BF16, name="cosT")
        sinTs = const.tile([P, S], BF16, name="sinTs")
        masks = const.tile([P, 3 * P], BF16, name="masks")
        perm_sb = const.tile([P, P], BF16, name="perm_sb")

        # warmup source: tiny memset lands ~0.7us, long before any DMA
        nc.gpsimd.memset(warm[:], 0.001)

        def chunked(dram, w):
            return dram.ap().rearrange("(c p) w -> p c w", p=P)

        xhv = xh_sb.rearrange("p (c w) -> p c w", c=CH)
        xlv = xl_sb.rearrange("p (c w) -> p c w", c=CH)
        # weights on scalar, x hi/lo chunk-pairs interleaved on sync: the
        # merged QK pass consumes each pair for q+k, m0+m1 at once
        nc.scalar.dma_start(wqkh_sb.rearrange("p (c w) -> p c w", c=CH),
                            chunked(wqkh_d, 2 * GD))
        nc.sync.dma_start(xh_sb[:, 0:2048], xh_d.ap()[0:P, 0:2048])
        nc.sync.dma_start(xl_sb[:, 0:2048], xl_d.ap()[0:P, 0:2048])
        nc.scalar.dma_start(wqkl_sb.rearrange("p (c w) -> p c w", c=CH),
                            chunked(wqkl_d, 2 * GD))
        for lo in range(1, 8):
            nc.sync.dma_start(xhv[:, lo:lo + 1],
                              chunked(xh_d, S)[:, lo:lo + 1])
            nc.sync.dma_start(xlv[:, lo:lo + 1],
                              chunked(xl_d, S)[:, lo:lo + 1])
        # trig/perm AFTER the x stream on sync: the x chunk-pairs pace the
        # m0 projection; rope needs these only ~5us later
        nc.sync.dma_start(cosT[:], cosT_d.ap())
        nc.sync.dma_start(sinTs[:], sinTs_d.ap())
        nc.sync.dma_start(perm_sb[:], perm_d.ap())

        def xc(hi, c):
            sb = xh_sb if hi else xl_sb
            return sb[:, c * S:(c + 1) * S]

        def wc(w_sb, c, width=GD):
            return w_sb[:, c * width:(c + 1) * width]

        qr_sb = [qk.tile([P, S], BF16, name=f"qr{m}") for m in range(2)]
        kr_sb = [qk.tile([P, S], BF16, name=f"kr{m}") for m in range(2)]
        q_sb = [qk.tile([P, S], BF16, name=f"q{m}") for m in range(2)]
        k_sb = [qk.tile([P, S], BF16, name=f"k{m}") for m in range(2)]
        shf_sb = [qk.tile([P, S], BF16, name=f"shf{m}{t}")
                  for m in range(2) for t in range(2)]
        v_sb = [vp.tile([P, HG * VW], BF16, name=f"v{t}") for t in range(NQT)]
        ctx_sb = [cxp.tile([P, S], BF16, name=f"cx{m}") for m in range(2)]
        _osb = [cxp.tile([P, H], BF16, tag="osb", name=f"ot{t}", bufs=10)
                for t in range(NQT)]

        def dr_mm3(ps, i, m, cp, n, term):
            """One term of the 3-term DoubleRow fp8 accumulation for
            chunk-pair cp of projection i (0=q, 1=k): term 0 Wh@xh,
            1 Wh@xl, 2 Wl@xh (K=256)."""
            c0 = 2 * cp
            w = (wqkh_sb if term < 2 else wqkl_sb).rearrange(
                "p (c w) -> p c w", c=CH)
            xv = xlv if term == 1 else xhv
            wo_ = i * GD + m * P
            DR = mybir.MatmulPerfMode.DoubleRow
            nc.tensor.matmul(
                ps[:, 0:512],
                w[:, c0:c0 + 2, wo_:wo_ + P],
                xv[:, c0:c0 + 2, n * 512:(n + 1) * 512],
                start=(cp == 0 and term == 0),
                stop=(cp == CP2 - 1 and term == 2),
                perf_mode=DR)

        def proj_drain(dest, m, n, ps, to_act):
            if to_act:
                nc.scalar.activation(
                    dest[m][:, n * 512:(n + 1) * 512], ps[:],
                    mybir.ActivationFunctionType.Copy,
                    scale=float(1.0 / WSCALE))
            else:
                nc.vector.tensor_scalar_mul(
                    dest[m][:, n * 512:(n + 1) * 512], ps[:],
                    float(1.0 / WSCALE))

        def rope(m, only=None):
            # rotate-half via 2 paired-block DMAs per tensor, then
            # 3 tensor_tensor ops (all 2x-eligible bf16) into fresh tiles.
            # The muls are emitted in 512-column chunks: the scheduler can
            # then release the early q-tiles' rotated data long before the
            # full 2048-wide tensor is done (the monolithic version ends
            # up serialized behind other DVE work and stalls the first
            # attention iterations).
            for ti, (src, dst) in enumerate(((q_sb, qr_sb), (k_sb, kr_sb))):
                if only is not None and ti != only:
                    continue
                shf = shf_sb[m * 2 + ti]
                for o in (0, HD):
                    nc.sync.dma_start(shf[o:o + 32, :], src[m][o + 32:o + 64, :])
                    nc.sync.dma_start(shf[o + 32:o + 64, :], src[m][o:o + 32, :])
                for ck in range(4):
                    cs = slice(ck * 512, (ck + 1) * 512)
                    nc.vector.tensor_mul(shf[:, cs], shf[:, cs],
                                         sinTs[:, cs])
                    nc.vector.tensor_mul(dst[m][:, cs], src[m][:, cs],
                                         cosT[:, cs])
                    nc.vector.tensor_add(dst[m][:, cs], dst[m][:, cs],
                                         shf[:, cs])

        def rope_pe_chunk(m, ti, n, pool):
            # rotate-half on the PE (permutation matmul) + Act psum drain:
            # no SP-queue/HWDGE round trip (which the scheduler serializes
            # far too late), and per-512-col chunks release early q-tiles
            src, dst = ((q_sb, qr_sb), (k_sb, kr_sb))[ti]
            shf = shf_sb[m * 2 + ti]
            cs = slice(n * 512, (n + 1) * 512)
            ps = pool.tile([P, 512], F32, tag="pj", name=f"rot{m}{ti}{n}")
            nc.tensor.matmul(ps[:], perm_sb[:], src[m][:, cs],
                             start=True, stop=True)
            nc.scalar.copy(shf[:, cs], ps[:])
            nc.vector.tensor_mul(shf[:, cs], shf[:, cs], sinTs[:, cs])
            nc.vector.tensor_mul(dst[m][:, cs], src[m][:, cs],
                                 cosT[:, cs])
            nc.vector.tensor_add(dst[m][:, cs], dst[m][:, cs],
                                 shf[:, cs])

        def attn_scores(sp, mt, qi):
            # baseline-proven layout: one [128, 1152] tile per (mt, hb) set,
            # block order [diag, far?, middles...] so the masked edges are
            # first and exp covers a contiguous prefix
            kt0 = max(0, qi - WT)
            nkt = qi - kt0 + 1
            kts = [qi]
            n_edge = 1
            if qi >= WT:
                kts.append(kt0)
                n_edge = 2
            kts.extend(range(kt0 + (1 if qi >= WT else 0), qi))

            prb = []
            for hb in range(2):
                ho = hb * HD
                s_ps = sp.tile([P, (WT + 1) * P], F32, tag="sp",
                               name=f"sps{mt}{qi}{hb}")
                for i, kt in enumerate(kts):
                    nc.tensor.matmul(
                        s_ps[:, i * P:(i + 1) * P],
                        kr_sb[mt][ho:ho + HD, kt * P:(kt + 1) * P],
                        qr_sb[mt][ho:ho + HD, qi * P:(qi + 1) * P],
                        start=True, stop=True)
                probs = pp.tile([P, (WT + 1) * P], BF16, tag="pp",
                                name=f"pr{mt}{qi}{hb}")
                nc.scalar.activation(
                    probs[:, 0:nkt * P], s_ps[:, 0:nkt * P],
                    mybir.ActivationFunctionType.Exp,
                    scale=float(1.0 / np.sqrt(HD)))
                nc.vector.tensor_mul(
                    probs[:, 0:n_edge * P], probs[:, 0:n_edge * P],
                    masks[:, 0:n_edge * P])
                prb.append((probs, 0))
            return kts, n_edge, prb

        def attn_ctx(cp, mt, qi, kts, n_edge, prb):
            nkt = len(kts)
            ctx_ps = cp.tile([VW, 2 * P], F32, tag="ctx", name=f"cps{mt}{qi}")
            for hb in range(2):
                h = mt * 2 + hb
                issue = list(range(n_edge, nkt)) + list(range(n_edge))
                pt, po_ = prb[hb]
                for j, i in enumerate(issue):
                    nc.tensor.matmul(
                        ctx_ps[:, hb * P:(hb + 1) * P],
                        v_sb[kts[i]][:, h * VW:(h + 1) * VW],
                        pt[:, po_ + i * P:po_ + (i + 1) * P],
                        start=(j == 0), stop=(j == nkt - 1))
            # start the normalization chain; the muls are emitted later
            # (norm_muls) so the DVE queue never blocks on the Pool
            # broadcast round-trip
            rinv = sm.tile([1, 2 * P], F32, tag="rinv", name=f"ri{mt}{qi}")
            nc.vector.reciprocal(rinv[:], ctx_ps[HD:HD + 1, 0:2 * P])
            rbc = sm.tile([HD, 2 * P], F32, tag="rbc", name=f"rb{mt}{qi}")
            nc.gpsimd.partition_broadcast(rbc[:], rinv[:])
            return ctx_ps, rbc

        def norm_muls(mt, qi, ctx_ps, rbc):
            cs = qi * P
            nc.vector.tensor_mul(ctx_sb[mt][0:HD, cs:cs + P],
                                 ctx_ps[0:HD, 0:P], rbc[:, 0:P])
            nc.vector.tensor_mul(ctx_sb[mt][HD:2 * HD, cs:cs + P],
                                 ctx_ps[0:HD, P:2 * P], rbc[:, P:2 * P])

        def outproj_t(cp, t, tail=False):
            # tail=True (last tile): two parallel drains (DVE + Act, both
            # idle by then) and ONE store — several small stores would
            # serialize 625ns apart on the single-slot HWDGE
            o_sb = _osb[t]
            pss = []
            for n in range(2):
                ps = cp.tile([P, 512], F32, tag="ctx", name=f"po{t}{n}")
                for c in range(2):
                    nc.tensor.matmul(
                        ps[:], ctx_sb[c][:, t * P:(t + 1) * P],
                        wc(wo_sb, c, H)[:, n * 512:(n + 1) * 512],
                        start=(c == 0), stop=(c == 1))
                pss.append(ps)
                if not tail:
                    dst = o_sb[:, n * 512:(n + 1) * 512]
                    nc.vector.tensor_copy(dst, ps[:])
                    nc.sync.dma_start(
                        out_d.ap()[t * P:(t + 1) * P,
                                   n * 512:(n + 1) * 512],
                        dst)
            if tail:
                nc.vector.tensor_copy(o_sb[:, 0:512], pss[0][:])
                nc.scalar.copy(o_sb[:, 512:1024], pss[1][:])
                nc.sync.dma_start(
                    out_d.ap()[t * P:(t + 1) * P, 0:1024], o_sb[:, 0:1024])

        # ---- phase 1: projections + V + rope ----
        with tc.tile_pool(name="pj", bufs=8, space="PSUM") as pj:
            # PE warmup: dummy matmuls on the memset tile keep the PE busy
            # from ~0.8us so (a) the p-state ramp completes before real work
            # and (b) the initial DMA latency is hidden. The warm psum tile
            # is allocated first so it lands in the pj bank written LAST by
            # the projection (no WAR stall).
            wps = pj.tile([P, 512], F32, tag="pj", name="warm_ps")
            for i in range(18):
                w = 512 if i < 5 else 256
                nc.tensor.matmul(wps[:, 0:w], warm[0:1, 0:128],
                                 warm[0:1, 128:128 + w],
                                 start=True, stop=True)
            # m0 pass: chunk-pair-outer, q/k interleaved over 8 psum
            # banks — keeps pace with the streaming x load. Terms emitted
            # hi-sweep-first so the lo weights may arrive later without
            # stalling the in-order PE.
            dests = (q_sb, k_sb)
            pss = {}
            for i in range(2):
                for n in range(4):
                    pss[(i, n)] = pj.tile([P, 512], F32, tag="pj",
                                          name=f"pj0{i}{n}")
            for cp in range(CP2):
                for i in range(2):
                    for n in range(4):
                        for term in range(3):
                            dr_mm3(pss[(i, n)], i, 0, cp, n, term)
            for (i, n), ps in pss.items():
                proj_drain(dests[i], 0, n, ps, to_act=True)
            rope(0)
            # m1 passes: pair-inner, one psum bank at a time, with the
            # PE-based rope chunks woven in per drained column block
            for i in range(2):
                for n in range(4):
                    ps = pj.tile([P, 512], F32, tag="pj", name=f"pj1{i}{n}")
                    for cp in range(CP2):
                        for term in range(3):
                            dr_mm3(ps, i, 1, cp, n, term)
                    # all m1 drains on Act: the DVE is busy with rope
                    # muls here, and the rope-rot matmuls wait on these
                    proj_drain(dests[i], 1, n, ps, to_act=True)
                for n in range(4):
                    rope_pe_chunk(1, i, n, pj)
            # on sync AFTER the x stream: keeps the x chunks first in
            # line on the shared DMA engines
            nc.sync.dma_start(wvh_sb.rearrange("p (c w) -> p c w", c=CH),
                              chunked(wvh_d, GD))
            nc.sync.dma_start(wvl_sb.rearrange("p (c w) -> p c w", c=CH),
                              chunked(wvl_d, GD))
            nc.sync.dma_start(wo_sb.rearrange("p (c w) -> p c w", c=2),
                              chunked(woT_d, H))
            nc.sync.dma_start(masks[:], masks_d.ap())
            for t in range(NQT):
                ones = v_sb[t].rearrange("p (h v) -> p h v", h=HG)[:, :, HD:VW]
                nc.gpsimd.memset(ones, 1.0)
            for t in range(NQT):
                ps = pj.tile([P, GD], F32, tag="pj", name=f"pjv{t}")
                for cp in range(CP2):
                    c0 = 2 * cp
                    DR = mybir.MatmulPerfMode.DoubleRow
                    lhs_h = xhv[:, c0:c0 + 2, t * P:(t + 1) * P]
                    lhs_l = xlv[:, c0:c0 + 2, t * P:(t + 1) * P]
                    wh = wvh_sb.rearrange("p (c w) -> p c w", c=CH)
                    wl = wvl_sb.rearrange("p (c w) -> p c w", c=CH)
                    rhs_h = wh[:, c0:c0 + 2, :]
                    rhs_l = wl[:, c0:c0 + 2, :]
                    nc.tensor.matmul(ps[:], lhs_h, rhs_h,
                                     start=(cp == 0), stop=False,
                                     perf_mode=DR)
                    nc.tensor.matmul(ps[:], lhs_l, rhs_h,
                                     start=False, stop=False, perf_mode=DR)
                    nc.tensor.matmul(ps[:], lhs_h, rhs_l,
                                     start=False, stop=(cp == CP2 - 1),
                                     perf_mode=DR)
                vdst = v_sb[t].rearrange("p (h v) -> p h v", h=HG)[:, :, 0:HD]
                vsrc = ps.rearrange("p (h d) -> p h d", h=HG)
                # all V drains on Act: it idles in late phase 1, while the
                # DVE must chew through the 12 rope muls before attention
                # can start
                nc.scalar.activation(
                    vdst, vsrc, mybir.ActivationFunctionType.Copy,
                    scale=float(1.0 / WSCALE))

        # ---- phase 2: interleaved attention + outproj ----
        order = list(range(NQT))
        OP_LAG = 2
        with tc.tile_pool(name="sp", bufs=2, space="PSUM") as sp, \
             tc.tile_pool(name="cp", bufs=2, space="PSUM") as cp:
            pend = [None, None]
            pqi = [None, None]
            nrm = [None, None]
            for p, qi in enumerate(order):
                for mt in range(2):
                    if nrm[mt] is not None:
                        norm_muls(mt, *nrm[mt])
                        nrm[mt] = None
                for mt in range(2):
                    cur = attn_scores(sp, mt, qi)
                    if pend[mt] is not None:
                        nrm[mt] = (pqi[mt],
                                   *attn_ctx(cp, mt, pqi[mt], *pend[mt]))
                    pend[mt] = cur
                    pqi[mt] = qi
                    if mt == 0 and p >= OP_LAG + 1:
                        # outproj between the two head-pairs: its PE time
                        # buys the Act exp stream slack before scores(mt1)
                        # hit the scores-psum ring WAR
                        outproj_t(cp, order[p - OP_LAG - 1])
            for mt in range(2):
                if nrm[mt] is not None:
                    norm_muls(mt, *nrm[mt])
                nrm[mt] = (pqi[mt], *attn_ctx(cp, mt, pqi[mt], *pend[mt]))
            for mt in range(2):
                norm_muls(mt, *nrm[mt])
            for p in range(len(order) - OP_LAG - 1, len(order)):
                outproj_t(cp, order[p], tail=True)

    nc.compile()
    return nc


def _rope_tables():
    inv_freq = 1.0 / (ROPE_THETA ** (np.arange(0, HD, 2, dtype=np.float64) / HD))
    t = np.arange(MAX_POS, dtype=np.float64)
    freqs = np.outer(t, inv_freq)
    emb = np.concatenate([freqs, freqs], axis=-1)
    return np.cos(emb).astype(np.float32), np.sin(emb).astype(np.float32)


def _split8(a):
    """Split float32 array into (hi e4m3, lo e5m2) so hi+lo ~ a."""
    hi = a.astype(ml_dtypes.float8_e4m3)
    lo = (a - hi.astype(np.float32)).astype(ml_dtypes.float8_e5m2)
    return hi, lo


def kernel(hidden_states, position_ids, wq, bq, wk, bk, wv, bv, wo, bo):
    bf16 = ml_dtypes.bfloat16
    if "nc" not in _cache:
        _cache["nc"] = _build()
    nc = _cache["nc"]

    assert not np.any(np.asarray(bq)) and not np.any(np.asarray(bk)), \
        "kernel assumes zero q/k biases (per problem spec)"

    cos_t, sin_t = _rope_tables()
    pos = np.clip(np.asarray(position_ids), 0, MAX_POS - 1).astype(np.int64)

    maskd = np.triu(np.ones((P, P), np.float32))
    maskf = np.tril(np.ones((P, P), np.float32), -1)
    masks = np.concatenate([maskd, maskf, maskf], axis=1).astype(bf16)

    # rotate-half permutation (sign lives in sinTs): out[d] = src[d^32]
    # within each 64-dim head block
    perm = np.zeros((P, P), np.float32)
    for blk in (0, HD):
        for d in range(32):
            perm[blk + d + 32, blk + d] = 1.0
            perm[blk + d, blk + d + 32] = 1.0
    perm = perm.astype(bf16)

    xh = [None] * B
    xl = [None] * B
    for b in range(B):
        xh[b], xl[b] = _split8(
            np.ascontiguousarray(np.asarray(hidden_states)[b].T,
                                 dtype=np.float32))

    in_maps = []
    for core in range(N_CORES):
        b, g = core // HG, core % HG
        sl = slice(g * GD, (g + 1) * GD)
        cos_b = cos_t[pos[b]]
        sin_b = sin_t[pos[b]]
        cosT = np.tile(cos_b.T, (2, 1)).astype(bf16)
        sin_sgn = sin_b.T.copy()
        sin_sgn[0:32] *= -1.0
        sinTs = np.tile(sin_sgn, (2, 1)).astype(bf16)
        wqk = np.concatenate([np.asarray(wq)[sl].T, np.asarray(wk)[sl].T],
                             axis=1).astype(np.float32) * WSCALE
        wqkh, wqkl = _split8(np.ascontiguousarray(wqk))
        wvh, wvl = _split8(np.ascontiguousarray(
            np.asarray(wv)[sl].T, dtype=np.float32) * WSCALE)
        in_maps.append({
            "xh": xh[b], "xl": xl[b],
            "wqkh": wqkh, "wqkl": wqkl,
            "wvh": wvh, "wvl": wvl,
            "woT": np.ascontiguousarray(np.asarray(wo)[:, sl].T).astype(bf16),
            "cosT": cosT,
            "sinTs": sinTs,
            "masks": masks,
            "perm": perm,
        })

    res = run_bass_kernel_spmd(nc, in_maps, core_ids=list(range(N_CORES)))

    const_off = (np.asarray(wo) @ np.asarray(bv) + np.asarray(bo)).astype(
        np.float32)
    out = np.empty((B, S, H), dtype=np.float32)
    for b in range(B):
        acc = res.results[b * HG]["out"].astype(np.float32)
        for g in range(1, HG):
            acc += res.results[b * HG + g]["out"].astype(np.float32)
        out[b] = acc + const_off[None, :]
    return out


# revision 109
# speedup vs baseline: 1.0005x; 1.0005x over previous
"""Sliding-window (banded causal) MHA on 8 TRN2 NeuronCores — v4.

Sharding: 8 cores = 2 batches x 4 head-groups (4 heads x 64 dims).
Baseline-proven interleaved attention loop, plus:
  - PE warmup: dummy matmuls from ~0.8us absorb the p-state ramp and
    the initial DMA latency (the ramp model halves throughput for the
    first ~3us of PE-busy otherwise).
  - fp8 DoubleRow projections: Q/K/V projections run as 3-term
    hi/lo-residual fp8 matmuls (Wh@xh + Wh@xl + Wl@xh accumulated in
    PSUM). K=256 per instruction at 0.5 cycles/row -> 0.75x the bf16
    row count for the same math, ~bf16 accuracy (hi=e4m3, lo=e5m2
    residuals; dropped Wl@xl term is ~0.13%).

Engine assignment (cost-model driven):
  PE    : projections, scores, PV, output projection
  Act   : exp ONLY (plus Q/K PSUM drains during the projection phase)
  DVE   : rope multiplies, probs edge masks, reciprocal, ctx drains,
          half the outproj drains
  Pool  : V drains, denominator partition-broadcast, other half of the
          outproj drains
  DMA   : loads, rope rotate-half shifts, stores (bf16)

Normalization: V carries a ones column, so PV emits raw ctx plus the
softmax sums (psum row 64). reciprocal (DVE, psum-in) -> partition_
broadcast (Pool) -> tensor_mul drains (DVE) fold the divide into the
PSUM drain. Biases: bq/bk are zero per the problem spec (asserted on
host); bv/bo are folded exactly into a host-side constant.
"""

from contextlib import ExitStack

import numpy as np
import ml_dtypes

import concourse.bass as bass
import concourse.tile as tile
from concourse import bacc, mybir
from concourse.bass_utils import run_bass_kernel_spmd

BF16 = mybir.dt.bfloat16
F32 = mybir.dt.float32
FP8H = mybir.dt.float8e4
FP8L = mybir.dt.float8e5

B, S, H = 2, 2048, 1024
NH, HD = 16, 64
WINDOW = 1024
ROPE_THETA = 10000.0
MAX_POS = 2048
N_CORES = 8
HG = 4                      # heads per core
GD = HG * HD                # 256
P = 128
NQT = S // P                # 16
WT = WINDOW // P            # 8
CH = H // P                 # 8
CP2 = CH // 2               # 4 chunk-pairs (K=256 per DoubleRow matmul)
VW = HD + 1                 # 65
WSCALE = 32.0               # host-side weight scale (fp8 subnormal guard)

_cache = {}


def _build():
    nc = bacc.Bacc("TRN2", target_bir_lowering=False, debug=False,
                   enable_asserts=False, num_devices=N_CORES)

    xh_d = nc.dram_tensor("xh", [H, S], FP8H, kind="ExternalInput")
    xl_d = nc.dram_tensor("xl", [H, S], FP8L, kind="ExternalInput")
    wqkh_d = nc.dram_tensor("wqkh", [H, 2 * GD], FP8H, kind="ExternalInput")
    wqkl_d = nc.dram_tensor("wqkl", [H, 2 * GD], FP8L, kind="ExternalInput")
    wvh_d = nc.dram_tensor("wvh", [H, GD], FP8H, kind="ExternalInput")
    wvl_d = nc.dram_tensor("wvl", [H, GD], FP8L, kind="ExternalInput")
    woT_d = nc.dram_tensor("woT", [GD, H], BF16, kind="ExternalInput")
    perm_d = nc.dram_tensor("perm", [P, P], BF16, kind="ExternalInput")
    cosT_d = nc.dram_tensor("cosT", [P, S], BF16, kind="ExternalInput")
    sinTs_d = nc.dram_tensor("sinTs", [P, S], BF16, kind="ExternalInput")
    masks_d = nc.dram_tensor("masks", [P, 3 * P], BF16, kind="ExternalInput")
    out_d = nc.dram_tensor("out", [S, H], BF16, kind="ExternalOutput")

    with tile.TileContext(nc) as tc, ExitStack() as ctx:
        const = ctx.enter_context(tc.tile_pool(name="const", bufs=1))
        qk = ctx.enter_context(tc.tile_pool(name="qk", bufs=1))
        vp = ctx.enter_context(tc.tile_pool(name="vp", bufs=1))
        pp = ctx.enter_context(tc.tile_pool(name="pp", bufs=20))
        cxp = ctx.enter_context(tc.tile_pool(name="cxp", bufs=1))
        sm = ctx.enter_context(tc.tile_pool(name="sm", bufs=10))

        warm = const.tile([1, 640], BF16, name="warm")
        wqkh_sb = const.tile([P, CH * 2 * GD], FP8H, name="wqkh_sb")
        wqkl_sb = const.tile([P, CH * 2 * GD], FP8L, name="wqkl_sb")
        wvh_sb = const.tile([P, CH * GD], FP8H, name="wvh_sb")
        wvl_sb = const.tile([P, CH * GD], FP8L, name="wvl_sb")
        wo_sb = const.tile([P, 2 * H], BF16, name="wo_sb")
        xh_sb = const.tile([P, CH * S], FP8H, name="xh_sb")
        xl_sb = const.tile([P, CH * S], FP8L, name="xl_sb")
        cosT = const.tile([P, S], BF16, name="cosT")
        sinTs = const.tile([P, S], BF16, name="sinTs")
        masks = const.tile([P, 3 * P], BF16, name="masks")
        perm_sb = const.tile([P, P], BF16, name="perm_sb")

        # warmup source: tiny memset lands ~0.7us, long before any DMA
        nc.gpsimd.memset(warm[:], 0.001)

        def chunked(dram, w):
            return dram.ap().rearrange("(c p) w -> p c w", p=P)

        xhv = xh_sb.rearrange("p (c w) -> p c w", c=CH)
        xlv = xl_sb.rearrange("p (c w) -> p c w", c=CH)
        # weights on scalar, x hi/lo chunk-pairs interleaved on sync: the
        # merged QK pass consumes each pair for q+k, m0+m1 at once
        nc.scalar.dma_start(wqkh_sb.rearrange("p (c w) -> p c w", c=CH),
                            chunked(wqkh_d, 2 * GD))
        nc.sync.dma_start(xh_sb[:, 0:2048], xh_d.ap()[0:P, 0:2048])
        nc.sync.dma_start(xl_sb[:, 0:2048], xl_d.ap()[0:P, 0:2048])
        nc.scalar.dma_start(wqkl_sb.rearrange("p (c w) -> p c w", c=CH),
                            chunked(wqkl_d, 2 * GD))
        for lo in range(1, 8):
            nc.sync.dma_start(xhv[:, lo:lo + 1],
                              chunked(xh_d, S)[:, lo:lo + 1])
            nc.sync.dma_start(xlv[:, lo:lo + 1],
                              chunked(xl_d, S)[:, lo:lo + 1])
        # trig/perm AFTER the x stream on sync: the x chunk-pairs pace the
        # m0 projection; rope needs these only ~5us later
        nc.sync.dma_start(cosT[:], cosT_d.ap())
        nc.sync.dma_start(sinTs[:], sinTs_d.ap())
        nc.sync.dma_start(perm_sb[:], perm_d.ap())

        def xc(hi, c):
            sb = xh_sb if hi else xl_sb
            return sb[:, c * S:(c + 1) * S]

        def wc(w_sb, c, width=GD):
            return w_sb[:, c * width:(c + 1) * width]

        qr_sb = [qk.tile([P, S], BF16, name=f"qr{m}") for m in range(2)]
        kr_sb = [qk.tile([P, S], BF16, name=f"kr{m}") for m in range(2)]
        q_sb = [qk.tile([P, S], BF16, name=f"q{m}") for m in range(2)]
        k_sb = [qk.tile([P, S], BF16, name=f"k{m}") for m in range(2)]
        shf_sb = [qk.tile([P, S], BF16, name=f"shf{m}{t}")
                  for m in range(2) for t in range(2)]
        v_sb = [vp.tile([P, HG * VW], BF16, name=f"v{t}") for t in range(NQT)]
        ctx_sb = [cxp.tile([P, S], BF16, name=f"cx{m}") for m in range(2)]
        _osb = [cxp.tile([P, H], BF16, tag="osb", name=f"ot{t}", bufs=10)
                for t in range(NQT)]

        def dr_mm3(ps, i, m, cp, n, term):
            """One term of the 3-term DoubleRow fp8 accumulation for
            chunk-pair cp of projection i (0=q, 1=k): term 0 Wh@xh,
            1 Wh@xl, 2 Wl@xh (K=256)."""
            c0 = 2 * cp
            w = (wqkh_sb if term < 2 else wqkl_sb).rearrange(
                "p (c w) -> p c w", c=CH)
            xv = xlv if term == 1 else xhv
            wo_ = i * GD + m * P
            DR = mybir.MatmulPerfMode.DoubleRow
            nc.tensor.matmul(
                ps[:, 0:512],
                w[:, c0:c0 + 2, wo_:wo_ + P],
                xv[:, c0:c0 + 2, n * 512:(n + 1) * 512],
                start=(cp == 0 and term == 0),
                stop=(cp == CP2 - 1 and term == 2),
                perf_mode=DR)

        def proj_drain(dest, m, n, ps, to_act):
            if to_act:
                nc.scalar.activation(
                    dest[m][:, n * 512:(n + 1) * 512], ps[:],
                    mybir.ActivationFunctionType.Copy,
                    scale=float(1.0 / WSCALE))
            else:
                nc.vector.tensor_scalar_mul(
                    dest[m][:, n * 512:(n + 1) * 512], ps[:],
                    float(1.0 / WSCALE))

        def rope(m, only=None):
            # rotate-half via 2 paired-block DMAs per tensor, then
            # 3 tensor_tensor ops (all 2x-eligible bf16) into fresh tiles.
            # The muls are emitted in 512-column chunks: the scheduler can
            # then release the early q-tiles' rotated data long before the
            # full 2048-wide tensor is done (the monolithic version ends
            # up serialized behind other DVE work and stalls the first
            # attention iterations).
            for ti, (src, dst) in enumerate(((q_sb, qr_sb), (k_sb, kr_sb))):
                if only is not None and ti != only:
                    continue
                shf = shf_sb[m * 2 + ti]
                for o in (0, HD):
                    nc.sync.dma_start(shf[o:o + 32, :], src[m][o + 32:o + 64, :])
                    nc.sync.dma_start(shf[o + 32:o + 64, :], src[m][o:o + 32, :])
                for ck in range(4):
                    cs = slice(ck * 512, (ck + 1) * 512)
                    nc.vector.tensor_mul(shf[:, cs], shf[:, cs],
                                         sinTs[:, cs])
                    nc.vector.tensor_mul(dst[m][:, cs], src[m][:, cs],
                                         cosT[:, cs])
                    nc.vector.tensor_add(dst[m][:, cs], dst[m][:, cs],
                                         shf[:, cs])

        def rope_pe_chunk(m, ti, n, pool):
            # rotate-half on the PE (permutation matmul) + Act psum drain:
            # no SP-queue/HWDGE round trip (which the scheduler serializes
            # far too late), and per-512-col chunks release early q-tiles
            src, dst = ((q_sb, qr_sb), (k_sb, kr_sb))[ti]
            shf = shf_sb[m * 2 + ti]
            cs = slice(n * 512, (n + 1) * 512)
            ps = pool.tile([P, 512], F32, tag="pj", name=f"rot{m}{ti}{n}")
            nc.tensor.matmul(ps[:], perm_sb[:], src[m][:, cs],
                             start=True, stop=True)
            nc.scalar.copy(shf[:, cs], ps[:])
            nc.vector.tensor_mul(shf[:, cs], shf[:, cs], sinTs[:, cs])
            nc.vector.tensor_mul(dst[m][:, cs], src[m][:, cs],
                                 cosT[:, cs])
            nc.vector.tensor_add(dst[m][:, cs], dst[m][:, cs],
                                 shf[:, cs])

        def attn_scores(sp, mt, qi):
            # baseline-proven layout: one [128, 1152] tile per (mt, hb) set,
            # block order [diag, far?, middles...] so the masked edges are
            # first and exp covers a contiguous prefix
            kt0 = max(0, qi - WT)
            nkt = qi - kt0 + 1
            kts = [qi]
            n_edge = 1
            if qi >= WT:
                kts.append(kt0)
                n_edge = 2
            kts.extend(range(kt0 + (1 if qi >= WT else 0), qi))

            prb = []
            for hb in range(2):
                ho = hb * HD
                s_ps = sp.tile([P, (WT + 1) * P], F32, tag="sp",
                               name=f"sps{mt}{qi}{hb}")
                for i, kt in enumerate(kts):
                    nc.tensor.matmul(
                        s_ps[:, i * P:(i + 1) * P],
                        kr_sb[mt][ho:ho + HD, kt * P:(kt + 1) * P],
                        qr_sb[mt][ho:ho + HD, qi * P:(qi + 1) * P],
                        start=True, stop=True)
                probs = pp.tile([P, (WT + 1) * P], BF16, tag="pp",
                                name=f"pr{mt}{qi}{hb}")
                nc.scalar.activation(
                    probs[:, 0:nkt * P], s_ps[:, 0:nkt * P],
                    mybir.ActivationFunctionType.Exp,
                    scale=float(1.0 / np.sqrt(HD)))
                nc.vector.tensor_mul(
                    probs[:, 0:n_edge * P], probs[:, 0:n_edge * P],
                    masks[:, 0:n_edge * P])
                prb.append((probs, 0))
            return kts, n_edge, prb

        def attn_ctx(cp, mt, qi, kts, n_edge, prb):
            nkt = len(kts)
            ctx_ps = cp.tile([VW, 2 * P], F32, tag="ctx", name=f"cps{mt}{qi}")
            for hb in range(2):
                h = mt * 2 + hb
                issue = list(range(n_edge, nkt)) + list(range(n_edge))
                pt, po_ = prb[hb]
                for j, i in enumerate(issue):
                    nc.tensor.matmul(
                        ctx_ps[:, hb * P:(hb + 1) * P],
                        v_sb[kts[i]][:, h * VW:(h + 1) * VW],
                        pt[:, po_ + i * P:po_ + (i + 1) * P],
                        start=(j == 0), stop=(j == nkt - 1))
            # start the normalization chain; the muls are emitted later
            # (norm_muls) so the DVE queue never blocks on the Pool
            # broadcast round-trip
            rinv = sm.tile([1, 2 * P], F32, tag="rinv", name=f"ri{mt}{qi}")
            nc.vector.reciprocal(rinv[:], ctx_ps[HD:HD + 1, 0:2 * P])
            rbc = sm.tile([HD, 2 * P], F32, tag="rbc", name=f"rb{mt}{qi}")
            nc.gpsimd.partition_broadcast(rbc[:], rinv[:])
            return ctx_ps, rbc

        def norm_muls(mt, qi, ctx_ps, rbc):
            cs = qi * P
            nc.vector.tensor_mul(ctx_sb[mt][0:HD, cs:cs + P],
                                 ctx_ps[0:HD, 0:P], rbc[:, 0:P])
            nc.vector.tensor_mul(ctx_sb[mt][HD:2 * HD, cs:cs + P],
                                 ctx_ps[0:HD, P:2 * P], rbc[:, P:2 * P])

        def outproj_t(cp, t, tail=False):
            # tail=True (last tile): two parallel drains (DVE + Act, both
            # idle by then) and ONE store — several small stores would
            # serialize 625ns apart on the single-slot HWDGE
            o_sb = _osb[t]
            pss = []
            for n in range(2):
                ps = cp.tile([P, 512], F32, tag="ctx", name=f"po{t}{n}")
                for c in range(2):
                    nc.tensor.matmul(
                        ps[:], ctx_sb[c][:, t * P:(t + 1) * P],
                        wc(wo_sb, c, H)[:, n * 512:(n + 1) * 512],
                        start=(c == 0), stop=(c == 1))
                pss.append(ps)
                if not tail:
                    dst = o_sb[:, n * 512:(n + 1) * 512]
                    nc.vector.tensor_copy(dst, ps[:])
                    nc.sync.dma_start(
                        out_d.ap()[t * P:(t + 1) * P,
                                   n * 512:(n + 1) * 512],
                        dst)
            if tail:
                nc.vector.tensor_copy(o_sb[:, 0:512], pss[0][:])
                nc.scalar.copy(o_sb[:, 512:1024], pss[1][:])
                nc.sync.dma_start(
                    out_d.ap()[t * P:(t + 1) * P, 0:1024], o_sb[:, 0:1024])

        # ---- phase 1: projections + V + rope ----
        with tc.tile_pool(name="pj", bufs=8, space="PSUM") as pj:
            # PE warmup: dummy matmuls on the memset tile keep the PE busy
            # from ~0.8us so (a) the p-state ramp completes before real work
            # and (b) the initial DMA latency is hidden. The warm psum tile
            # is allocated first so it lands in the pj bank written LAST by
            # the projection (no WAR stall).
            wps = pj.tile([P, 512], F32, tag="pj", name="warm_ps")
            for i in range(18):
                w = 512 if i < 5 else 256
                nc.tensor.matmul(wps[:, 0:w], warm[0:1, 0:128],
                                 warm[0:1, 128:128 + w],
                                 start=True, stop=True)
            # m0 pass: chunk-pair-outer, q/k interleaved over 8 psum
            # banks — keeps pace with the streaming x load. Terms emitted
            # hi-sweep-first so the lo weights may arrive later without
            # stalling the in-order PE.
            dests = (q_sb, k_sb)
            pss = {}
            for i in range(2):
                for n in range(4):
                    pss[(i, n)] = pj.tile([P, 512], F32, tag="pj",
                                          name=f"pj0{i}{n}")
            for cp in range(CP2):
                for i in range(2):
                    for n in range(4):
                        for term in range(3):
                            dr_mm3(pss[(i, n)], i, 0, cp, n, term)
            for (i, n), ps in pss.items():
                proj_drain(dests[i], 0, n, ps, to_act=True)
            rope(0)
            # m1 passes: pair-inner, one psum bank at a time, with the
            # PE-based rope chunks woven in per drained column block
            for i in range(2):
                for n in range(4):
                    ps = pj.tile([P, 512], F32, tag="pj", name=f"pj1{i}{n}")
                    for cp in range(CP2):
                        for term in range(3):
                            dr_mm3(ps, i, 1, cp, n, term)
                    # all m1 drains on Act: the DVE is busy with rope
                    # muls here, and the rope-rot matmuls wait on these
                    proj_drain(dests[i], 1, n, ps, to_act=True)
                for n in range(4):
                    rope_pe_chunk(1, i, n, pj)
            # on sync AFTER the x stream: keeps the x chunks first in
            # line on the shared DMA engines
            nc.sync.dma_start(wvh_sb.rearrange("p (c w) -> p c w", c=CH),
                              chunked(wvh_d, GD))
            nc.sync.dma_start(wvl_sb.rearrange("p (c w) -> p c w", c=CH),
                              chunked(wvl_d, GD))
            nc.sync.dma_start(wo_sb.rearrange("p (c w) -> p c w", c=2),
                              chunked(woT_d, H))
            nc.sync.dma_start(masks[:], masks_d.ap())
            for t in range(NQT):
                ones = v_sb[t].rearrange("p (h v) -> p h v", h=HG)[:, :, HD:VW]
                nc.gpsimd.memset(ones, 1.0)
            for t in range(NQT):
                ps = pj.tile([P, GD], F32, tag="pj", name=f"pjv{t}")
                for cp in range(CP2):
                    c0 = 2 * cp
                    DR = mybir.MatmulPerfMode.DoubleRow
                    lhs_h = xhv[:, c0:c0 + 2, t * P:(t + 1) * P]
                    lhs_l = xlv[:, c0:c0 + 2, t * P:(t + 1) * P]
                    wh = wvh_sb.rearrange("p (c w) -> p c w", c=CH)
                    wl = wvl_sb.rearrange("p (c w) -> p c w", c=CH)
                    rhs_h = wh[:, c0:c0 + 2, :]
                    rhs_l = wl[:, c0:c0 + 2, :]
                    nc.tensor.matmul(ps[:], lhs_h, rhs_h,
                                     start=(cp == 0), stop=False,
                                     perf_mode=DR)
                    nc.tensor.matmul(ps[:], lhs_l, rhs_h,
                                     start=False, stop=False, perf_mode=DR)
                    nc.tensor.matmul(ps[:], lhs_h, rhs_l,
                                     start=False, stop=(cp == CP2 - 1),
                                     perf_mode=DR)
                vdst = v_sb[t].rearrange("p (h v) -> p h v", h=HG)[:, :, 0:HD]
                vsrc = ps.rearrange("p (h d) -> p h d", h=HG)
                # V drains on Act while it idles in late phase 1 (the DVE
                # must chew through the rope muls); the LAST tiles go to
                # DVE (free by then), clearing Act for the first exps
                if t < 12:
                    nc.scalar.activation(
                        vdst, vsrc, mybir.ActivationFunctionType.Copy,
                        scale=float(1.0 / WSCALE))
                else:
                    nc.vector.tensor_scalar_mul(vdst, vsrc,
                                                float(1.0 / WSCALE))

        # ---- phase 2: interleaved attention + outproj ----
        order = list(range(NQT))
        OP_LAG = 2
        with tc.tile_pool(name="sp", bufs=2, space="PSUM") as sp, \
             tc.tile_pool(name="cp", bufs=2, space="PSUM") as cp:
            pend = [None, None]
            pqi = [None, None]
            nrm = [None, None]
            for p, qi in enumerate(order):
                for mt in range(2):
                    if nrm[mt] is not None:
                        norm_muls(mt, *nrm[mt])
                        nrm[mt] = None
                for mt in range(2):
                    cur = attn_scores(sp, mt, qi)
                    if pend[mt] is not None:
                        nrm[mt] = (pqi[mt],
                                   *attn_ctx(cp, mt, pqi[mt], *pend[mt]))
                    pend[mt] = cur
                    pqi[mt] = qi
                    if mt == 0 and p >= OP_LAG + 1:
                        # outproj between the two head-pairs: its PE time
                        # buys the Act exp stream slack before scores(mt1)
                        # hit the scores-psum ring WAR
                        outproj_t(cp, order[p - OP_LAG - 1])
            for mt in range(2):
                if nrm[mt] is not None:
                    norm_muls(mt, *nrm[mt])
                nrm[mt] = (pqi[mt], *attn_ctx(cp, mt, pqi[mt], *pend[mt]))
            for mt in range(2):
                norm_muls(mt, *nrm[mt])
            for p in range(len(order) - OP_LAG - 1, len(order)):
                outproj_t(cp, order[p], tail=True)

    nc.compile()
    return nc


def _rope_tables():
    inv_freq = 1.0 / (ROPE_THETA ** (np.arange(0, HD, 2, dtype=np.float64) / HD))
    t = np.arange(MAX_POS, dtype=np.float64)
    freqs = np.outer(t, inv_freq)
    emb = np.concatenate([freqs, freqs], axis=-1)
    return np.cos(emb).astype(np.float32), np.sin(emb).astype(np.float32)


def _split8(a):
    """Split float32 array into (hi e4m3, lo e5m2) so hi+lo ~ a."""
    hi = a.astype(ml_dtypes.float8_e4m3)
    lo = (a - hi.astype(np.float32)).astype(ml_dtypes.float8_e5m2)
    return hi, lo


def kernel(hidden_states, position_ids, wq, bq, wk, bk, wv, bv, wo, bo):
    bf16 = ml_dtypes.bfloat16
    if "nc" not in _cache:
        _cache["nc"] = _build()
    nc = _cache["nc"]

    assert not np.any(np.asarray(bq)) and not np.any(np.asarray(bk)), \
        "kernel assumes zero q/k biases (per problem spec)"

    cos_t, sin_t = _rope_tables()
    pos = np.clip(np.asarray(position_ids), 0, MAX_POS - 1).astype(np.int64)

    maskd = np.triu(np.ones((P, P), np.float32))
    maskf = np.tril(np.ones((P, P), np.float32), -1)
    masks = np.concatenate([maskd, maskf, maskf], axis=1).astype(bf16)

    # rotate-half permutation (sign lives in sinTs): out[d] = src[d^32]
    # within each 64-dim head block
    perm = np.zeros((P, P), np.float32)
    for blk in (0, HD):
        for d in range(32):
            perm[blk + d + 32, blk + d] = 1.0
            perm[blk + d, blk + d + 32] = 1.0
    perm = perm.astype(bf16)

    xh = [None] * B
    xl = [None] * B
    for b in range(B):
        xh[b], xl[b] = _split8(
            np.ascontiguousarray(np.asarray(hidden_states)[b].T,
                                 dtype=np.float32))

    in_maps = []
    for core in range(N_CORES):
        b, g = core // HG, core % HG
        sl = slice(g * GD, (g + 1) * GD)
        cos_b = cos_t[pos[b]]
        sin_b = sin_t[pos[b]]
        cosT = np.tile(cos_b.T, (2, 1)).astype(bf16)
        sin_sgn = sin_b.T.copy()
        sin_sgn[0:32] *= -1.0
        sinTs = np.tile(sin_sgn, (2, 1)).astype(bf16)
        wqk = np.concatenate([np.asarray(wq)[sl].T, np.asarray(wk)[sl].T],
                             axis=1).astype(np.float32) * WSCALE
        wqkh, wqkl = _split8(np.ascontiguousarray(wqk))
        wvh, wvl = _split8(np.ascontiguousarray(
            np.asarray(wv)[sl].T, dtype=np.float32) * WSCALE)
        in_maps.append({
            "xh": xh[b], "xl": xl[b],
            "wqkh": wqkh, "wqkl": wqkl,
            "wvh": wvh, "wvl": wvl,
            "woT": np.ascontiguousarray(np.asarray(wo)[:, sl].T).astype(bf16),
            "cosT": cosT,
            "sinTs": sinTs,
            "masks": masks,
            "perm": perm,
        })

    res = run_bass_kernel_spmd(nc, in_maps, core_ids=list(range(N_CORES)))

    const_off = (np.asarray(wo) @ np.asarray(bv) + np.asarray(bo)).astype(
        np.float32)
    out = np.empty((B, S, H), dtype=np.float32)
    for b in range(B):
        acc = res.results[b * HG]["out"].astype(np.float32)
        for g in range(1, HG):
            acc += res.results[b * HG + g]["out"].astype(np.float32)
        out[b] = acc + const_off[None, :]
    return out
